# revision 16
# baseline (speedup 1.0000x reference)
"""Trainium2 Bass kernel for DifferentiableSoftmaxTree NLL (hierarchical
softmax negative log-likelihood).

Math: the 2-way log_softmax at each tree node reduces to a softplus of a
logit difference, so for sample b with path nodes n_k / directions d_k:
    s_k  = features[b] . (node_weights[n_k,:,1] - node_weights[n_k,:,0])
    out[b] = sum_k mask_k * softplus((1-2 d_k) * s_k)

Strategy (data-parallel over batch, 8 cores x 512 samples, 4 blocks of 128):

  TOP LEVELS (0..8, heap node ids 0..510): every sample visits all 9 of
  these levels, so the device matmuls the block's features against ALL
  511 node weight-diff columns on the PE (fp16, 4 contraction chunks of
  128 into one PSUM bank -> [128,511] fp32 logits). The host sends a
  signed multi-hot mask mh[b,n] = sign_j at the 9 path nodes (0
  elsewhere); one DVE tensor_tensor against PSUM yields u = sign*logit
  at path nodes and exactly 0 elsewhere. softplus(0)=ln2 is constant, so
  a per-sample correction (NU - pathlen)*ln2 fixes the sum -- no
  per-level masking on device.

  DEEP LEVELS (9..15): the HOST pre-gathers each sample's 7 deep-level
  weight-diff rows into a dense [BL, 7*512] fp16 stream (a measured
  on-device SWDGE gather pays ~570ns of fixed cost PER ROW -> ~100GB/s;
  a dense HWDGE read of the same bytes runs at HBM line rate). Dot
  products vs features on DVE (fp16 multiply in-place + per-level
  reduction).

  All other per-sample operands (feat fp16, featT fp16 for the PE, mh
  fp16, meta) are packed into ONE [128, PKW] int32 row per sample so each
  block issues a single HWDGE DMA besides the deep stream.

  SOFTPLUS of all 518 terms in 2 ACT ops: Exp(u) then Ln(e+1) with
  accum_out summing the row (|u| <~ 12 here so exp can't overflow), then
  subtract the host correction. (Exp and Ln live in different ACT
  table-sets on this runtime -- each switch costs ~1.3us -- so a direct
  Softplus table is used instead when available.)

  (tensor_tensor_reduce is avoided: it wedges this runtime.)
"""

import numpy as np
from contextlib import ExitStack

import concourse.bass as bass
import concourse.mybir as mybir
import concourse.tile as tile
from concourse import bass_utils
import concourse.bacc as bacc

NUM_CLASSES = 50000
NUM_INTERNAL = NUM_CLASSES - 1
D = 512
B = 4096
K = 16
N_CORES = 8
BL = B // N_CORES          # samples per core
P = 128                    # partition dim
NBLK = BL // P             # 128-sample blocks per core
JTOP = 9                   # tree levels computed via PE matmul
NTOP = (1 << JTOP) - 1     # 511 heap nodes in levels 0..8
KD = K - JTOP              # 7 deep levels gathered per sample
NU = NTOP + KD             # softplus terms per sample
LN2 = float(np.log(2.0))

# tuning flags (settled by probe measurements):
# - Softplus activation table is a different function on this runtime
#   (probe: max abs err 36 vs log1p(exp)) -> Exp+Ln pair.
# - tensor_reduce runs at 1x mode regardless of dtype/shape (probe: 673ns
#   per [128,512] fp16 level, 3.87us for the 3D form) -> tree-fold the
#   512-wide levels down to 64 with 2x-mode fp16 tensor_tensor adds, then
#   one small 3D reduce.
USE_SOFTPLUS = False
FOLD_TO = 64               # level width after TT tree-folds (then 3D reduce)

_AF = mybir.ActivationFunctionType
_OP = mybir.AluOpType
_F16 = mybir.dt.float16
_F32 = mybir.dt.float32
_I32 = mybir.dt.int32

# packed per-sample input row, int32 words:
#   fp16[0:512)     = features
#   fp16[512:1024)  = featT block rows (d-major chunks for the PE)
#   fp16[1024:1535) = mh signed multi-hot over top nodes (fp16[1535] pad)
#   w769            = fp32 correction (NU - pathlen)*ln2
#   w770..776       = fp32 deep masked signs
PKW = 796


def _build_program():
    nc = bacc.Bacc(
        "TRN2",
        target_bir_lowering=False,
        debug=False,
        enable_asserts=False,
        num_devices=N_CORES,
    )
    pk_ap = nc.dram_tensor("pk", [BL, PKW], _I32, kind="ExternalInput").ap()
    pdeep_ap = nc.dram_tensor("pdeep", [BL, KD * D], _F16, kind="ExternalInput").ap()
    wtopT_ap = nc.dram_tensor("wtopT", [P, 4 * NTOP], _F16, kind="ExternalInput").ap()
    out_ap = nc.dram_tensor("out", [BL, 1], _F32, kind="ExternalOutput").ap()

    with tile.TileContext(nc) as tc, ExitStack() as ctx:
        const_pool = ctx.enter_context(tc.tile_pool(name="const", bufs=1))
        pk_pool = ctx.enter_context(tc.tile_pool(name="pk", bufs=NBLK))
        deep_pool = ctx.enter_context(tc.tile_pool(name="deep", bufs=NBLK))
        u_pool = ctx.enter_context(tc.tile_pool(name="u", bufs=NBLK))
        e_pool = ctx.enter_context(tc.tile_pool(name="e", bufs=NBLK))
        dump_pool = ctx.enter_context(tc.tile_pool(name="dump", bufs=2))
        small_pool = ctx.enter_context(tc.tile_pool(name="small", bufs=2 * NBLK))
        psum_pool = ctx.enter_context(tc.tile_pool(name="psum", bufs=2, space="PSUM"))

        # prefetch: issue every input DMA up front so all blocks stream
        # concurrently while the engine preambles run (pdeep on the ACT
        # HWDGE ring, the rest on the SP ring, to parallelize issue).
        pk_ts, g_ts = [], []
        for blk in range(NBLK):
            b0 = blk * P
            pk_t = pk_pool.tile([P, PKW], _I32, tag="pk")
            nc.sync.dma_start(pk_t[:], pk_ap[b0 : b0 + P, :])
            g_t = deep_pool.tile([P, KD * D], _F16, tag="g")
            nc.scalar.dma_start(g_t[:], pdeep_ap[b0 : b0 + P, :])
            pk_ts.append(pk_t)
            g_ts.append(g_t)
            if blk == 0:
                wt_t = const_pool.tile([P, 4 * NTOP], _F16, tag="wt")
                nc.sync.dma_start(wt_t[:], wtopT_ap[:])

        u_ts = []
        for blk in range(NBLK):
            b0 = blk * P
            pk_t = pk_ts[blk]
            g_t = g_ts[blk]

            pk16 = pk_t[:].bitcast(_F16)        # [P, 2*PKW]
            pk32 = pk_t[:].bitcast(_F32)        # [P, PKW]

            # all 511 top-level logits for the block: featT.T @ wtopT
            ps_t = psum_pool.tile([P, NTOP], _F32, tag="ps")
            for c in range(4):
                nc.tensor.matmul(
                    ps_t[:],
                    lhsT=pk16[:, 512 + c * P : 512 + (c + 1) * P],
                    rhs=wt_t[:, c * NTOP : (c + 1) * NTOP],
                    start=(c == 0),
                    stop=(c == 3),
                )

            u_t = u_pool.tile([P, NU], _F32, tag="u")
            nc.vector.tensor_tensor(
                out=u_t[:, 0:NTOP], in0=pk16[:, 1024 : 1024 + NTOP], in1=ps_t[:],
                op=_OP.mult,
            )

            # deep levels: in-place multiply by features, reduce per level
            g3 = g_t[:].rearrange("p (k d) -> p k d", k=KD)
            nc.vector.tensor_tensor(
                out=g3,
                in0=g3,
                in1=pk16[:, 0:D][:, None, :].to_broadcast([P, KD, D]),
                op=_OP.mult,
            )
            w = D
            while w > FOLD_TO:
                h = w // 2
                nc.vector.tensor_tensor(
                    out=g_t[:].rearrange("p (k d) -> p k d", k=KD)[:, :, 0:h],
                    in0=g_t[:].rearrange("p (k d) -> p k d", k=KD)[:, :, 0:h],
                    in1=g_t[:].rearrange("p (k d) -> p k d", k=KD)[:, :, h:w],
                    op=_OP.add,
                )
                w = h
            s_t = small_pool.tile([P, KD], _F32, tag="s")
            nc.vector.tensor_reduce(
                out=s_t[:],
                in_=g_t[:].rearrange("p (k d) -> p k d", k=KD)[:, :, 0:FOLD_TO],
                axis=mybir.AxisListType.X,
                op=_OP.add,
            )
            nc.vector.tensor_tensor(
                out=u_t[:, NTOP:NU], in0=s_t[:], in1=pk32[:, 770 : 770 + KD],
                op=_OP.mult,
            )
            u_ts.append(u_t)

        # sum softplus(u) = Exp then Ln(e+1) with accum_out. Exp and Ln live
        # in different ACT table-sets (~1.3us per switch), so batch all Exps
        # then all Lns: 2 table loads total instead of 8.
        e_ts = []
        for blk in range(NBLK):
            e_t = e_pool.tile([P, NU], _F32, tag="e")
            nc.scalar.activation(e_t[:], u_ts[blk][:], _AF.Exp)
            e_ts.append(e_t)
        for blk in range(NBLK):
            b0 = blk * P
            acc_t = small_pool.tile([P, 1], _F32, tag="acc")
            d_t = dump_pool.tile([P, NU], _F32, tag="d")
            nc.scalar.activation(
                d_t[:], e_ts[blk][:], _AF.Ln, bias=1.0, accum_out=acc_t[:]
            )
            res_t = small_pool.tile([P, 1], _F32, tag="res")
            nc.vector.tensor_scalar(
                out=res_t[:],
                in0=acc_t[:],
                scalar1=pk_ts[blk][:].bitcast(_F32)[:, 769:770],
                scalar2=None,
                op0=_OP.subtract,
            )
            nc.sync.dma_start(out_ap[b0 : b0 + P, :], res_t[:])

    nc.compile()
    return nc


_PROGRAM_CACHE = {}


def _get_program():
    if "nc" not in _PROGRAM_CACHE:
        _PROGRAM_CACHE["nc"] = _build_program()
    return _PROGRAM_CACHE["nc"]


def _reset_device():
    # A previously-crashed kernel can leave an exec unit wedged; a
    # client-side axon reset clears it and is near-free otherwise.
    try:
        import ctypes

        lib = ctypes.CDLL("/opt/axon/libaxon_pjrt.so")
        lib.axon_reset.restype = ctypes.c_int64
        lib.axon_reset()
    except Exception:
        pass


def _prepare_inputs(features, targets, node_weights, path_nodes_map, path_directions_map):
    features = np.asarray(features, dtype=np.float32)
    targets = np.asarray(targets, dtype=np.int32)
    node_weights = np.asarray(node_weights, dtype=np.float32)
    path_nodes_map = np.asarray(path_nodes_map, dtype=np.int32)
    path_directions_map = np.asarray(path_directions_map, dtype=np.int32)

    wdiff = node_weights[:, :, 1] - node_weights[:, :, 0]     # [N_INT, D] f32
    maskmap = path_nodes_map != -1                             # [C, K]
    wdiff16 = wdiff.astype(np.float16)

    # top-level weight matrix, chunked for the PE:
    # wtopT[p, c*NTOP + n] = wdiff[n, c*128 + p]
    wtopT = np.ascontiguousarray(
        wdiff16[:NTOP].reshape(NTOP, 4, P).transpose(2, 1, 0).reshape(P, 4 * NTOP)
    )

    # per-sample metadata
    tflat = targets.reshape(-1)
    bnodes = path_nodes_map[tflat]                             # [B, K]
    bdirs = path_directions_map[tflat]
    bmask = maskmap[tflat]
    pathlen = bmask.sum(axis=1).astype(np.int32)               # 15 or 16
    sgn = (1 - 2 * bdirs).astype(np.float32)                   # [B, K]
    msgn_deep = np.where(bmask[:, JTOP:], sgn[:, JTOP:], np.float32(0.0))
    corr = (NU - pathlen).astype(np.float32) * np.float32(LN2)

    # signed multi-hot over the 511 top nodes
    mh = np.zeros((B, NTOP + 1), dtype=np.float16)
    rows = np.arange(B)
    for j in range(JTOP):
        mh[rows, bnodes[:, j]] = sgn[:, j].astype(np.float16)
    mh = mh[:, :NTOP]

    # host pre-gather of each sample's deep-level rows (masked levels zero)
    deep_nodes = np.where(bmask[:, JTOP:], bnodes[:, JTOP:], 0)   # [B, KD]
    pdeep = wdiff16[deep_nodes]                                   # [B, KD, D]
    pdeep[~bmask[:, JTOP:]] = np.float16(0.0)
    pdeep = np.ascontiguousarray(pdeep.reshape(B, KD * D))

    feat16 = features.astype(np.float16)                          # [B, D]

    in_maps = []
    for i in range(N_CORES):
        sl = slice(i * BL, (i + 1) * BL)
        fc = feat16[sl]                                           # [BL, D]
        # featT[blk*128+p, c*128+i] = fc[blk*128+i, c*128+p]
        ftT = fc.reshape(NBLK, P, 4, P).transpose(0, 3, 2, 1).reshape(BL, D)

        pk = np.zeros((BL, PKW), dtype=np.int32)
        pk16 = pk.view(np.float16)                                # [BL, 2*PKW]
        pk32 = pk.view(np.float32)                                # [BL, PKW]
        pk16[:, 0:D] = fc
        pk16[:, D : 2 * D] = ftT
        pk16[:, 1024 : 1024 + NTOP] = mh[sl]
        pk32[:, 769] = corr[sl]
        pk32[:, 770 : 770 + KD] = msgn_deep[sl]

        in_maps.append(
            {
                "pk": np.ascontiguousarray(pk),
                "pdeep": pdeep[sl],
                "wtopT": wtopT,
            }
        )
    return in_maps


def kernel(features, targets, node_weights, path_nodes_map, path_directions_map):
    in_maps = _prepare_inputs(
        features, targets, node_weights, path_nodes_map, path_directions_map
    )
    _reset_device()
    nc = _get_program()
    res = bass_utils.run_bass_kernel_spmd(nc, in_maps, core_ids=list(range(N_CORES)))
    out = np.concatenate([res.results[i]["out"].reshape(-1) for i in range(N_CORES)])
    return out.astype(np.float32)


# revision 24
# speedup vs baseline: 1.0832x; 1.0832x over previous
"""Trainium2 Bass kernel for DifferentiableSoftmaxTree NLL (hierarchical
softmax negative log-likelihood).

Math: the 2-way log_softmax at each tree node reduces to a softplus of a
logit difference, so for sample b with path nodes n_k / directions d_k:
    s_k  = features[b] . (node_weights[n_k,:,1] - node_weights[n_k,:,0])
    out[b] = sum_k mask_k * softplus((1-2 d_k) * s_k)

Strategy (data-parallel over batch, 8 cores x 512 samples, 4 blocks of 128):

  TOP LEVELS (0..8, heap ids 0..510): every sample visits all 9, so the
  PE matmuls the block's features against ALL 511 weight-diff columns
  (fp16, 4 contraction chunks into one PSUM bank). ACT copies the fp32
  PSUM logits to SBUF as fp16; one 2x-mode DVE multiply against a
  host-built signed multi-hot (sign at the 9 path nodes, 0 elsewhere)
  yields u = sign*logit at path nodes, exactly 0 off-path. softplus(0)=
  ln2 is constant, folded into a host-side per-sample correction.

  DEEP LEVELS (9..15): the HOST pre-gathers each sample's 7 deep rows
  (an on-device SWDGE gather pays ~570ns fixed cost PER ROW ~ 100GB/s;
  the same bytes stream densely at ~280GB/s). Dot products on DVE:
  fp16 multiply (2x), tree-fold 512->64 with 2x TT adds (tensor_reduce
  only runs 1x on this runtime), then one small 3D reduce.

  ONE DMA per block: deep rows + features + featT (d-major for the PE) +
  multi-hot + meta are packed in a single [128, 10352B] fp16 row on the
  ACT HWDGE ring (big per-partition descriptors; the 3KB pk rows of the
  previous revision crawled at 8.7GB/s/descriptor). All block DMAs are
  issued up front and stream while the engine preambles run.

  SOFTPLUS: Exp then Ln(e+1, accum_out=row sum). Exp and Ln live in
  different ACT table-sets (1.54us per switch), and the scheduler
  re-interleaves independent per-block activations, so blocks are PAIRED
  through one fused [128, 2*NU] Exp tile: 4 table loads instead of 8.
  The final subtraction of the correction runs on ACT as Identity with
  bias = -corr (per-partition AP), keeping the output chain off DVE.

  (tensor_tensor_reduce wedges this runtime; gpsimd elementwise ops
  starve DVE via the shared SBUF ports -- both measured, both avoided.)
"""

import numpy as np
from contextlib import ExitStack

import concourse.bass as bass
import concourse.mybir as mybir
import concourse.tile as tile
from concourse import bass_utils
import concourse.bacc as bacc

NUM_CLASSES = 50000
NUM_INTERNAL = NUM_CLASSES - 1
D = 512
B = 4096
K = 16
N_CORES = 8
BL = B // N_CORES          # samples per core
P = 128                    # partition dim
NBLK = BL // P             # 128-sample blocks per core
JTOP = 9                   # tree levels computed via PE matmul
NTOP = (1 << JTOP) - 1     # 511 heap nodes in levels 0..8
KD = K - JTOP              # 7 deep levels per sample
NU = NTOP + KD             # softplus terms per sample
LN2 = float(np.log(2.0))
FOLD_TO = 64               # level width after TT tree-folds

_AF = mybir.ActivationFunctionType
_OP = mybir.AluOpType
_F16 = mybir.dt.float16
_F32 = mybir.dt.float32

# packed per-sample row, fp16 elements:
#   [0:3584)      deep-level weight rows (levels 9..15, masked rows zero)
#   [3584:4096)   features
#   [4096:4608)   featT block rows (d-major chunks for the PE)
#   [4608:5119)   mh signed multi-hot over top nodes ([5119] pad)
#   fp32 w2560    -corr = -(NU - pathlen)*ln2  (ACT Identity bias)
#   fp32 w2561..7 deep masked signs
PDW = 5136                 # fp16 elements per row (10272 B)
OF_FEAT = KD * D           # 3584
OF_FTT = OF_FEAT + D       # 4096
OF_MH = OF_FTT + D         # 4608
OF_META32 = (OF_MH + NTOP + 1) // 2  # fp32 word index 2560


def _build_program():
    nc = bacc.Bacc(
        "TRN2",
        target_bir_lowering=False,
        debug=False,
        enable_asserts=False,
        num_devices=N_CORES,
    )
    pd_ap = nc.dram_tensor("pd", [BL, PDW], _F16, kind="ExternalInput").ap()
    wtopT_ap = nc.dram_tensor("wtopT", [P, 4 * NTOP], _F16, kind="ExternalInput").ap()
    out_ap = nc.dram_tensor("out", [BL, 1], _F32, kind="ExternalOutput").ap()

    with tile.TileContext(nc) as tc, ExitStack() as ctx:
        const_pool = ctx.enter_context(tc.tile_pool(name="const", bufs=1))
        pd_pool = ctx.enter_context(tc.tile_pool(name="pd", bufs=NBLK))
        lg_pool = ctx.enter_context(tc.tile_pool(name="lg", bufs=2))
        u_pool = ctx.enter_context(tc.tile_pool(name="u", bufs=2))
        e_pool = ctx.enter_context(tc.tile_pool(name="e", bufs=2))
        dump_pool = ctx.enter_context(tc.tile_pool(name="dump", bufs=2))
        small_pool = ctx.enter_context(tc.tile_pool(name="small", bufs=2 * NBLK))
        psum_pool = ctx.enter_context(tc.tile_pool(name="psum", bufs=2, space="PSUM"))

        # prefetch everything up front on the ACT HWDGE ring
        wt_t = const_pool.tile([P, 4 * NTOP], _F16, tag="wt")
        nc.scalar.dma_start(wt_t[:], wtopT_ap[:])
        pd_ts = []
        for blk in range(NBLK):
            b0 = blk * P
            pd_t = pd_pool.tile([P, PDW], _F16, tag="pd")
            nc.scalar.dma_start(pd_t[:], pd_ap[b0 : b0 + P, :])
            pd_ts.append(pd_t)

        # pair blocks through one fused Exp tile to pin the ACT table order
        upair_ts = []
        for _pair in range(NBLK // 2):
            upair_t = u_pool.tile([P, 2 * NU], _F16, tag="u")
            upair_ts.append(upair_t)

        for blk in range(NBLK):
            pd_t = pd_ts[blk]
            pd32 = pd_t[:].bitcast(_F32)        # [P, PDW/2]
            u_t = upair_ts[blk // 2]
            uo = (blk % 2) * NU                 # this block's offset in the pair

            # all 511 top-level logits: featT.T @ wtopT -> PSUM
            ps_t = psum_pool.tile([P, NTOP], _F32, tag="ps")
            for c in range(4):
                nc.tensor.matmul(
                    ps_t[:],
                    lhsT=pd_t[:, OF_FTT + c * P : OF_FTT + (c + 1) * P],
                    rhs=wt_t[:, c * NTOP : (c + 1) * NTOP],
                    start=(c == 0),
                    stop=(c == 3),
                )
            # ACT copies PSUM -> SBUF fp16 so the DVE multiply runs 2x
            lg_t = lg_pool.tile([P, NTOP], _F16, tag="lg")
            nc.scalar.activation(lg_t[:], ps_t[:], _AF.Copy)
            nc.vector.tensor_tensor(
                out=u_t[:, uo : uo + NTOP],
                in0=pd_t[:, OF_MH : OF_MH + NTOP],
                in1=lg_t[:],
                op=_OP.mult,
            )

            # deep levels: in-place multiply by features, fold, reduce
            g3 = pd_t[:, 0 : KD * D].rearrange("p (k d) -> p k d", k=KD)
            nc.vector.tensor_tensor(
                out=g3,
                in0=g3,
                in1=pd_t[:, OF_FEAT : OF_FEAT + D][:, None, :].to_broadcast(
                    [P, KD, D]
                ),
                op=_OP.mult,
            )
            w = D
            while w > FOLD_TO:
                h = w // 2
                nc.vector.tensor_tensor(
                    out=g3[:, :, 0:h], in0=g3[:, :, 0:h], in1=g3[:, :, h:w],
                    op=_OP.add,
                )
                w = h
            s_t = small_pool.tile([P, KD], _F32, tag="s")
            nc.vector.tensor_reduce(
                out=s_t[:], in_=g3[:, :, 0:FOLD_TO],
                axis=mybir.AxisListType.X, op=_OP.add,
            )
            nc.vector.tensor_tensor(
                out=u_t[:, uo + NTOP : uo + NU], in0=s_t[:],
                in1=pd32[:, OF_META32 + 1 : OF_META32 + 1 + KD],
                op=_OP.mult,
            )

        # softplus sums: per pair ONE Exp over [P, 2*NU], then two Ln+accum
        for pair in range(NBLK // 2):
            e_t = e_pool.tile([P, 2 * NU], _F32, tag="e")
            nc.scalar.activation(e_t[:], upair_ts[pair][:], _AF.Exp)
            for half in range(2):
                blk = 2 * pair + half
                b0 = blk * P
                acc_t = small_pool.tile([P, 1], _F32, tag="acc")
                d_t = dump_pool.tile([P, NU], _F32, tag="d")
                nc.scalar.activation(
                    d_t[:], e_t[:, half * NU : (half + 1) * NU], _AF.Ln,
                    bias=1.0, accum_out=acc_t[:],
                )
                res_t = small_pool.tile([P, 1], _F32, tag="res")
                nc.scalar.activation(
                    res_t[:], acc_t[:], _AF.Identity,
                    bias=pd_ts[blk][:].bitcast(_F32)[:, OF_META32 : OF_META32 + 1],
                )
                nc.sync.dma_start(out_ap[b0 : b0 + P, :], res_t[:])

    nc.compile()
    return nc


_PROGRAM_CACHE = {}


def _get_program():
    if "nc" not in _PROGRAM_CACHE:
        _PROGRAM_CACHE["nc"] = _build_program()
    return _PROGRAM_CACHE["nc"]


def _reset_device():
    # A previously-crashed kernel can leave an exec unit wedged; a
    # client-side axon reset clears it and is near-free otherwise.
    try:
        import ctypes

        lib = ctypes.CDLL("/opt/axon/libaxon_pjrt.so")
        lib.axon_reset.restype = ctypes.c_int64
        lib.axon_reset()
    except Exception:
        pass


def _prepare_inputs(features, targets, node_weights, path_nodes_map, path_directions_map):
    features = np.asarray(features, dtype=np.float32)
    targets = np.asarray(targets, dtype=np.int32)
    node_weights = np.asarray(node_weights, dtype=np.float32)
    path_nodes_map = np.asarray(path_nodes_map, dtype=np.int32)
    path_directions_map = np.asarray(path_directions_map, dtype=np.int32)

    wdiff = node_weights[:, :, 1] - node_weights[:, :, 0]     # [N_INT, D] f32
    maskmap = path_nodes_map != -1                             # [C, K]
    wdiff16 = wdiff.astype(np.float16)

    # top-level weight matrix, chunked for the PE:
    # wtopT[p, c*NTOP + n] = wdiff[n, c*128 + p]
    wtopT = np.ascontiguousarray(
        wdiff16[:NTOP].reshape(NTOP, 4, P).transpose(2, 1, 0).reshape(P, 4 * NTOP)
    )

    # per-sample metadata
    tflat = targets.reshape(-1)
    bnodes = path_nodes_map[tflat]                             # [B, K]
    bdirs = path_directions_map[tflat]
    bmask = maskmap[tflat]
    pathlen = bmask.sum(axis=1).astype(np.int32)               # 15 or 16
    sgn = (1 - 2 * bdirs).astype(np.float32)                   # [B, K]
    msgn_deep = np.where(bmask[:, JTOP:], sgn[:, JTOP:], np.float32(0.0))
    ncorr = -(NU - pathlen).astype(np.float32) * np.float32(LN2)

    # signed multi-hot over the 511 top nodes
    mh = np.zeros((B, NTOP + 1), dtype=np.float16)
    rows = np.arange(B)
    for j in range(JTOP):
        mh[rows, bnodes[:, j]] = sgn[:, j].astype(np.float16)

    # host pre-gather of each sample's deep-level rows (masked levels zero)
    deep_nodes = np.where(bmask[:, JTOP:], bnodes[:, JTOP:], 0)   # [B, KD]
    pdeep = wdiff16[deep_nodes]                                   # [B, KD, D]
    pdeep[~bmask[:, JTOP:]] = np.float16(0.0)

    feat16 = features.astype(np.float16)                          # [B, D]

    in_maps = []
    for i in range(N_CORES):
        sl = slice(i * BL, (i + 1) * BL)
        fc = feat16[sl]                                           # [BL, D]
        # featT[blk*128+p, c*128+i] = fc[blk*128+i, c*128+p]
        ftT = fc.reshape(NBLK, P, 4, P).transpose(0, 3, 2, 1).reshape(BL, D)

        pd = np.zeros((BL, PDW), dtype=np.float16)
        pd[:, 0 : KD * D] = pdeep[sl].reshape(BL, KD * D)
        pd[:, OF_FEAT : OF_FEAT + D] = fc
        pd[:, OF_FTT : OF_FTT + D] = ftT
        pd[:, OF_MH : OF_MH + NTOP + 1] = mh[sl]
        pd32 = pd.view(np.float32)
        pd32[:, OF_META32] = ncorr[sl]
        pd32[:, OF_META32 + 1 : OF_META32 + 1 + KD] = msgn_deep[sl]

        in_maps.append(
            {"pd": np.ascontiguousarray(pd), "wtopT": wtopT}
        )
    return in_maps


def kernel(features, targets, node_weights, path_nodes_map, path_directions_map):
    in_maps = _prepare_inputs(
        features, targets, node_weights, path_nodes_map, path_directions_map
    )
    _reset_device()
    nc = _get_program()
    res = bass_utils.run_bass_kernel_spmd(nc, in_maps, core_ids=list(range(N_CORES)))
    out = np.concatenate([res.results[i]["out"].reshape(-1) for i in range(N_CORES)])
    return out.astype(np.float32)


# revision 30
# speedup vs baseline: 1.1927x; 1.1011x over previous
"""Trainium2 Bass kernel for DifferentiableSoftmaxTree NLL (hierarchical
softmax negative log-likelihood).

Math: the 2-way log_softmax at each tree node reduces to a softplus of a
logit difference, so for sample b with path nodes n_k / directions d_k:
    s_k  = features[b] . (node_weights[n_k,:,1] - node_weights[n_k,:,0])
    out[b] = sum_k mask_k * softplus((1-2 d_k) * s_k)

Strategy (data-parallel over batch, 8 cores x 512 samples, 4 blocks of 128):

  TOP LEVELS (0..8, heap ids 0..510): every sample visits all 9, so the
  PE matmuls the block's features against ALL 511 weight-diff columns
  (fp16, 4 contraction chunks into one PSUM bank). ACT copies the fp32
  PSUM logits to SBUF as fp16; one 2x-mode DVE multiply against a
  host-built signed multi-hot (sign at the 9 path nodes, 0 elsewhere)
  yields u = sign*logit at path nodes, exactly 0 off-path. softplus(0)=
  ln2 is constant, folded into a host-side per-sample correction.

  DEEP LEVELS (9..15): the HOST pre-gathers each sample's 7 deep rows
  (an on-device SWDGE gather pays ~570ns fixed cost PER ROW ~ 100GB/s;
  the same bytes stream densely at ~280GB/s). Dot products on DVE:
  fp16 multiply (2x), tree-fold 512->64 with 2x TT adds (tensor_reduce
  only runs 1x on this runtime), then one small 3D reduce.

  ONE DMA per block: deep rows + features + featT (d-major for the PE) +
  multi-hot + meta are packed in a single [128, 10352B] fp16 row on the
  ACT HWDGE ring (big per-partition descriptors; the 3KB pk rows of the
  previous revision crawled at 8.7GB/s/descriptor). All block DMAs are
  issued up front and stream while the engine preambles run.

  SOFTPLUS: Exp then Ln(e+1, accum_out=row sum). Exp and Ln live in
  different ACT table-sets (1.54us per switch), and the scheduler
  re-interleaves independent per-block activations, so blocks are PAIRED
  through one fused [128, 2*NU] Exp tile: 4 table loads instead of 8.
  The final subtraction of the correction runs on ACT as Identity with
  bias = -corr (per-partition AP), keeping the output chain off DVE.

  (tensor_tensor_reduce wedges this runtime; gpsimd elementwise ops
  starve DVE via the shared SBUF ports -- both measured, both avoided.)
"""

import numpy as np
from contextlib import ExitStack

import concourse.bass as bass
import concourse.mybir as mybir
import concourse.tile as tile
from concourse import bass_utils
import concourse.bacc as bacc

NUM_CLASSES = 50000
NUM_INTERNAL = NUM_CLASSES - 1
D = 512
B = 4096
K = 16
N_CORES = 8
BL = B // N_CORES          # samples per core
P = 128                    # partition dim
NBLK = BL // P             # 128-sample blocks per core
JTOP = 9                   # tree levels computed via PE matmul
NTOP = (1 << JTOP) - 1     # 511 heap nodes in levels 0..8
KD = K - JTOP              # 7 deep levels per sample
NU = NTOP + KD             # softplus terms per sample
LN2 = float(np.log(2.0))
FOLD_TO = 32               # level width after TT tree-folds

_AF = mybir.ActivationFunctionType
_OP = mybir.AluOpType
_F16 = mybir.dt.float16
_F32 = mybir.dt.float32

# packed per-sample row, fp16 elements:
#   [0:3584)      deep-level weight rows (levels 9..15, masked rows zero)
#   [3584:4096)   features
#   [4096:4608)   featT block rows (d-major chunks for the PE)
#   [4608:5119)   mh signed multi-hot over top nodes ([5119] pad)
#   fp32 w2560    -corr = -(NU - pathlen)*ln2  (ACT Identity bias)
#   fp32 w2561..7 deep masked signs
PDW = 5136                 # fp16 elements per row (10272 B)
OF_FEAT = KD * D           # 3584
OF_FTT = OF_FEAT + D       # 4096
OF_MH = OF_FTT + D         # 4608
OF_META32 = (OF_MH + NTOP + 1) // 2  # fp32 word index 2560


def _build_program():
    nc = bacc.Bacc(
        "TRN2",
        target_bir_lowering=False,
        debug=False,
        enable_asserts=False,
        num_devices=N_CORES,
    )
    pd_ap = nc.dram_tensor("pd", [BL, PDW], _F16, kind="ExternalInput").ap()
    wtopT_ap = nc.dram_tensor("wtopT", [P, 4 * NTOP], _F16, kind="ExternalInput").ap()
    # [partition, block] layout -- ONE tail DMA; the host untransposes.
    out_ap = nc.dram_tensor("out", [P, NBLK], _F32, kind="ExternalOutput").ap()

    with tile.TileContext(nc) as tc, ExitStack() as ctx:
        const_pool = ctx.enter_context(tc.tile_pool(name="const", bufs=1))
        pd_pool = ctx.enter_context(tc.tile_pool(name="pd", bufs=NBLK))
        lg_pool = ctx.enter_context(tc.tile_pool(name="lg", bufs=2))
        u_pool = ctx.enter_context(tc.tile_pool(name="u", bufs=2))
        e_pool = ctx.enter_context(tc.tile_pool(name="e", bufs=2))
        dump_pool = ctx.enter_context(tc.tile_pool(name="dump", bufs=2))
        small_pool = ctx.enter_context(tc.tile_pool(name="small", bufs=2 * NBLK))
        psum_pool = ctx.enter_context(tc.tile_pool(name="psum", bufs=2, space="PSUM"))

        # prefetch everything up front on the ACT HWDGE ring (block 0 first
        # so DVE compute starts as early as possible)
        pd_ts = []
        for blk in range(NBLK):
            b0 = blk * P
            pd_t = pd_pool.tile([P, PDW], _F16, tag="pd")
            nc.scalar.dma_start(pd_t[:], pd_ap[b0 : b0 + P, :])
            pd_ts.append(pd_t)
            if blk == 0:
                wt_t = const_pool.tile([P, 4 * NTOP], _F16, tag="wt")
                nc.scalar.dma_start(wt_t[:], wtopT_ap[:])

        # ACT grouping: blocks 0+1 share one fused Exp tile (pins the table
        # order: Exp once, Ln twice); blocks 2 and 3 run solo so the
        # end-of-kernel ACT chain after the last DVE op is just Exp+Ln+res.
        # groups[g] = (u_tile, [block ids])
        u01_t = u_pool.tile([P, 2 * NU], _F16, tag="u01")
        u2_t = u_pool.tile([P, NU], _F16, tag="u2")
        u3_t = u_pool.tile([P, NU], _F16, tag="u3")
        groups = [(u01_t, [0, 1]), (u2_t, [2]), (u3_t, [3])]
        ublk = {0: (u01_t, 0), 1: (u01_t, NU), 2: (u2_t, 0), 3: (u3_t, 0)}

        res_t = small_pool.tile([P, NBLK], _F32, tag="res")

        for blk in range(NBLK):
            pd_t = pd_ts[blk]
            pd32 = pd_t[:].bitcast(_F32)        # [P, PDW/2]
            u_t, uo = ublk[blk]

            # all 511 top-level logits: featT.T @ wtopT -> PSUM
            ps_t = psum_pool.tile([P, NTOP], _F32, tag="ps")
            for c in range(4):
                nc.tensor.matmul(
                    ps_t[:],
                    lhsT=pd_t[:, OF_FTT + c * P : OF_FTT + (c + 1) * P],
                    rhs=wt_t[:, c * NTOP : (c + 1) * NTOP],
                    start=(c == 0),
                    stop=(c == 3),
                )
            # ACT copies PSUM -> SBUF fp16 so the DVE multiply runs 2x
            lg_t = lg_pool.tile([P, NTOP], _F16, tag="lg")
            nc.scalar.activation(lg_t[:], ps_t[:], _AF.Copy)
            nc.vector.tensor_tensor(
                out=u_t[:, uo : uo + NTOP],
                in0=pd_t[:, OF_MH : OF_MH + NTOP],
                in1=lg_t[:],
                op=_OP.mult,
            )

            # deep levels: in-place multiply by features, fold, reduce
            g3 = pd_t[:, 0 : KD * D].rearrange("p (k d) -> p k d", k=KD)
            nc.vector.tensor_tensor(
                out=g3,
                in0=g3,
                in1=pd_t[:, OF_FEAT : OF_FEAT + D][:, None, :].to_broadcast(
                    [P, KD, D]
                ),
                op=_OP.mult,
            )
            w = D
            while w > FOLD_TO:
                h = w // 2
                nc.vector.tensor_tensor(
                    out=g3[:, :, 0:h], in0=g3[:, :, 0:h], in1=g3[:, :, h:w],
                    op=_OP.add,
                )
                w = h
            s_t = small_pool.tile([P, KD], _F32, tag="s")
            nc.vector.tensor_reduce(
                out=s_t[:], in_=g3[:, :, 0:FOLD_TO],
                axis=mybir.AxisListType.X, op=_OP.add,
            )
            nc.vector.tensor_tensor(
                out=u_t[:, uo + NTOP : uo + NU], in0=s_t[:],
                in1=pd32[:, OF_META32 + 1 : OF_META32 + 1 + KD],
                op=_OP.mult,
            )

        # softplus sums: per group ONE Exp, then Ln+accum and the correction
        # subtraction (Identity with bias=-corr) per block, all on ACT
        for u_t, blks in groups:
            e_t = e_pool.tile([P, len(blks) * NU], _F32, tag="e")
            nc.scalar.activation(e_t[:], u_t[:], _AF.Exp)
            for half, blk in enumerate(blks):
                acc_t = small_pool.tile([P, 1], _F32, tag="acc")
                d_t = dump_pool.tile([P, NU], _F32, tag="d")
                nc.scalar.activation(
                    d_t[:], e_t[:, half * NU : (half + 1) * NU], _AF.Ln,
                    bias=1.0, accum_out=acc_t[:],
                )
                nc.scalar.activation(
                    res_t[:, blk : blk + 1], acc_t[:], _AF.Identity,
                    bias=pd_ts[blk][:].bitcast(_F32)[:, OF_META32 : OF_META32 + 1],
                )
        nc.sync.dma_start(out_ap[:], res_t[:])

    nc.compile()
    return nc


_PROGRAM_CACHE = {}


def _get_program():
    if "nc" not in _PROGRAM_CACHE:
        _PROGRAM_CACHE["nc"] = _build_program()
    return _PROGRAM_CACHE["nc"]


def _reset_device():
    # A previously-crashed kernel can leave an exec unit wedged; a
    # client-side axon reset clears it and is near-free otherwise.
    try:
        import ctypes

        lib = ctypes.CDLL("/opt/axon/libaxon_pjrt.so")
        lib.axon_reset.restype = ctypes.c_int64
        lib.axon_reset()
    except Exception:
        pass


def _prepare_inputs(features, targets, node_weights, path_nodes_map, path_directions_map):
    features = np.asarray(features, dtype=np.float32)
    targets = np.asarray(targets, dtype=np.int32)
    node_weights = np.asarray(node_weights, dtype=np.float32)
    path_nodes_map = np.asarray(path_nodes_map, dtype=np.int32)
    path_directions_map = np.asarray(path_directions_map, dtype=np.int32)

    wdiff = node_weights[:, :, 1] - node_weights[:, :, 0]     # [N_INT, D] f32
    maskmap = path_nodes_map != -1                             # [C, K]
    wdiff16 = wdiff.astype(np.float16)

    # top-level weight matrix, chunked for the PE:
    # wtopT[p, c*NTOP + n] = wdiff[n, c*128 + p]
    wtopT = np.ascontiguousarray(
        wdiff16[:NTOP].reshape(NTOP, 4, P).transpose(2, 1, 0).reshape(P, 4 * NTOP)
    )

    # per-sample metadata
    tflat = targets.reshape(-1)
    bnodes = path_nodes_map[tflat]                             # [B, K]
    bdirs = path_directions_map[tflat]
    bmask = maskmap[tflat]
    pathlen = bmask.sum(axis=1).astype(np.int32)               # 15 or 16
    sgn = (1 - 2 * bdirs).astype(np.float32)                   # [B, K]
    msgn_deep = np.where(bmask[:, JTOP:], sgn[:, JTOP:], np.float32(0.0))
    ncorr = -(NU - pathlen).astype(np.float32) * np.float32(LN2)

    # signed multi-hot over the 511 top nodes
    mh = np.zeros((B, NTOP + 1), dtype=np.float16)
    rows = np.arange(B)
    for j in range(JTOP):
        mh[rows, bnodes[:, j]] = sgn[:, j].astype(np.float16)

    # host pre-gather of each sample's deep-level rows (masked levels zero)
    deep_nodes = np.where(bmask[:, JTOP:], bnodes[:, JTOP:], 0)   # [B, KD]
    pdeep = wdiff16[deep_nodes]                                   # [B, KD, D]
    pdeep[~bmask[:, JTOP:]] = np.float16(0.0)

    feat16 = features.astype(np.float16)                          # [B, D]

    in_maps = []
    for i in range(N_CORES):
        sl = slice(i * BL, (i + 1) * BL)
        fc = feat16[sl]                                           # [BL, D]
        # featT[blk*128+p, c*128+i] = fc[blk*128+i, c*128+p]
        ftT = fc.reshape(NBLK, P, 4, P).transpose(0, 3, 2, 1).reshape(BL, D)

        pd = np.zeros((BL, PDW), dtype=np.float16)
        pd[:, 0 : KD * D] = pdeep[sl].reshape(BL, KD * D)
        pd[:, OF_FEAT : OF_FEAT + D] = fc
        pd[:, OF_FTT : OF_FTT + D] = ftT
        pd[:, OF_MH : OF_MH + NTOP + 1] = mh[sl]
        pd32 = pd.view(np.float32)
        pd32[:, OF_META32] = ncorr[sl]
        pd32[:, OF_META32 + 1 : OF_META32 + 1 + KD] = msgn_deep[sl]

        in_maps.append(
            {"pd": np.ascontiguousarray(pd), "wtopT": wtopT}
        )
    return in_maps


def kernel(features, targets, node_weights, path_nodes_map, path_directions_map):
    in_maps = _prepare_inputs(
        features, targets, node_weights, path_nodes_map, path_directions_map
    )
    _reset_device()
    nc = _get_program()
    res = bass_utils.run_bass_kernel_spmd(nc, in_maps, core_ids=list(range(N_CORES)))
    # device output is [partition, block]; sample b = blk*128 + p
    out = np.concatenate(
        [res.results[i]["out"].T.reshape(-1) for i in range(N_CORES)]
    )
    return out.astype(np.float32)
